# revision 1
# baseline (speedup 1.0000x reference)
"""CRF loss kernel v3 for Trainium2: fp8 one-hots + DoubleRow count matmuls.

Same layout as v2 (position = 256p + k, 257-column mtag with boundary col 0),
but the one-hot store h8 is a persistent [128, 257, 128] float8e4 tile:
  - emit mm (per tile k):   ps_e += h8[k+1]^T @ E_k    (fp8 lhsT x bf16 rhs)
  - count mm (per even k):  ps_c += DoubleRow(lhsT=h8[k+1:k+3], rhs=h8[k:k+2])
    -- one fp8 DoubleRow matmul counts TWO tile-pairs at 0.5 cyc/row,
    cutting count-matmul PE time 4x (verified exact on HW).
PE drops from 27.3k to ~17.1k ns; one-hot generation is spread across
three engines (group modes: AC = DVE bf16 one-hots + Act batch-convert
to fp8, PD = Pool direct fp8, DD = DVE direct fp8), leaving the kernel
paced by the 8MB/core bf16 emission DMA stream (~24.1k ns busy).
"""
import sys
import json

for p in ('/opt/trn_rl_repo', '/opt/trn_rl_repo/concourse'):
    if p not in sys.path:
        sys.path.insert(0, p)

import numpy as np

B, S, T = 512, 512, 128
NCORES = 8
BSH = B // NCORES
NPOS = BSH * S                 # 32768 positions per core
KT = NPOS // 128               # 256 tiles
G = 8
NEG = KT // G                  # 32 E groups
NHG = NEG + 1                  # 33 one-hot groups (257 one-hots)


def _mode(g):
    # DD: DVE direct fp8; PD: Pool direct fp8; AC: DVE bf16 + Act convert;
    # PC: DVE bf16 + Pool convert
    if g < 4:
        return 'DD'
    return ['PD', 'AC', 'AC', 'PD', 'AC', 'DD', 'AC'][(g - 4) % 7]


def _split_waits_json(bir_bytes: bytes, max_waits: int = 1) -> bytes:
    d = json.loads(bir_bytes)
    ctr = 0
    for f in d['functions']:
        for blk in f['blocks']:
            insts = blk.get('instructions')
            if not insts:
                continue
            out = []
            changed = False
            for ins in insts:
                si = ins.get('sync_info')
                if si and len(si.get('on_wait') or []) > max_waits:
                    waits = si['on_wait']
                    for w in waits[:-max_waits]:
                        ctr += 1
                        nop = {'engine': ins['engine'], 'ins': [], 'outs': [],
                               'name': f'wsplit-{ctr}', 'opcode': 'NoOp',
                               'sync_info': {'on_wait': [w], 'on_update': []}}
                        if 'debug' in ins:
                            nop['debug'] = ins['debug']
                        out.append(nop)
                    si['on_wait'] = waits[-max_waits:]
                    changed = True
                out.append(ins)
            if changed:
                blk['instructions'] = out
    return json.dumps(d).encode()


_patched = False


def _install_patch(bass_module):
    global _patched
    if _patched:
        return
    _patched = True
    orig = bass_module.Bass.to_json_bytes

    def patched(self):
        return _split_waits_json(orig(self))

    bass_module.Bass.to_json_bytes = patched


def _build():
    import concourse.bass as bass
    import concourse.mybir as mybir
    import concourse.tile as tile
    from concourse.masks import make_identity
    _install_patch(bass)
    f32 = mybir.dt.float32
    bf16 = mybir.dt.bfloat16
    fp8 = mybir.dt.float8e4
    i32 = mybir.dt.int32
    Alu = mybir.AluOpType
    DR = mybir.MatmulPerfMode.DoubleRow

    nc = bass.Bass()
    em = nc.dram_tensor('em', [NPOS, T], bf16, kind='ExternalInput')
    mtag = nc.dram_tensor('mtag', [128, KT + 1], bf16, kind='ExternalInput')
    tr = nc.dram_tensor('tr', [T, T], f32, kind='ExternalInput')
    out = nc.dram_tensor('out', [128, 3], f32, kind='ExternalOutput')

    em_r = em.rearrange("(p k) t -> p k t", k=KT)

    with tile.TileContext(nc) as tc:
        with tc.tile_pool(name='per', bufs=1) as per, \
             tc.tile_pool(name='eblk', bufs=5) as eblk, \
             tc.tile_pool(name='h16p', bufs=8) as h16p, \
             tc.tile_pool(name='ps', bufs=1, space='PSUM') as psp:

            # startup: one LONG first transfer (full E0) so the following
            # hwdge+dge latency chains hide under it; tags second
            e_first = eblk.tile([128, G, 128], bf16, tag='e')
            nc.sync.dma_start(out=e_first, in_=em_r[:, 0:G, :])
            mtag_b = per.tile([128, KT + 1], bf16)
            nc.sync.dma_start(out=mtag_b, in_=mtag[:, :])

            iota_i = per.tile([128, 128], i32)
            nc.gpsimd.iota(iota_i, pattern=[[1, 128]], base=0, channel_multiplier=0)
            iota_b = per.tile([128, 128], bf16)
            nc.vector.tensor_copy(iota_b, iota_i)
            mtag_sb = per.tile([128, KT + 1], f32)
            nc.vector.tensor_copy(mtag_sb, mtag_b)

            idt = per.tile([128, 256], f32)    # [identity | transitions^T]
            make_identity(nc, idt[:, 0:128])

            red = per.tile([128, 3], f32)
            msk_scr = per.tile([128, KT], bf16)
            scr = per.tile([128, 256], f32)

            h8 = per.tile([128, KT + 1, 128], fp8)   # persistent one-hot store

            ps_e = psp.tile([128, 128], f32)
            ps_c = psp.tile([128, 128], f32)

            next_ck = 0    # next even k whose DoubleRow count mm to issue
            e_tiles = {}
            LAG = 2        # groups of one-hot lookahead before matmuls consume
            for g in range(NHG + LAG):
                if g < NHG:
                    if g == 0:
                        e_tiles[0] = e_first
                    elif g < NEG and g % 2 == 1:
                        # E DMA groups of 16 tiles (2 one-hot groups)
                        ew = 2 if g + 1 < NEG else 1
                        e_blk = eblk.tile([128, 2 * G, 128], bf16, tag='e')
                        nc.sync.dma_start(out=e_blk[:, 0:ew * G, :],
                                          in_=em_r[:, g * G:(g + ew) * G, :])
                        e_tiles[g] = e_blk
                        if ew == 2:
                            e_tiles[g + 1] = None
                    if g == 3:
                        nc.sync.dma_start(out=idt[:, 128:256], in_=tr[:, :])
                    nt = G if g < NEG else 0
                    mode = _mode(g) if nt == G else 'DD'
                    if mode == 'DD':
                        idxs = list(range(g * G, g * G + nt)) + ([KT] if g == 0 else [])
                        for i in idxs:
                            nc.vector.tensor_scalar(out=h8[:, i, :], in0=iota_b,
                                                    scalar1=mtag_sb[:, i:i + 1],
                                                    scalar2=None, op0=Alu.is_equal)
                    elif mode in ('PD', 'HP'):
                        half = nt // 2 if mode == 'HP' else nt
                        for i in range(g * G, g * G + half):
                            nc.gpsimd.tensor_scalar(out=h8[:, i, :], in0=iota_b,
                                                    scalar1=mtag_sb[:, i:i + 1],
                                                    scalar2=None, op0=Alu.is_equal)
                        for i in range(g * G + half, g * G + nt):
                            nc.vector.tensor_scalar(out=h8[:, i, :], in0=iota_b,
                                                    scalar1=mtag_sb[:, i:i + 1],
                                                    scalar2=None, op0=Alu.is_equal)
                    else:   # AC / PC: DVE bf16 one-hots, then batch convert
                        h16 = h16p.tile([128, G, 128], bf16, tag='h16')
                        for j in range(nt):
                            i = g * G + j
                            nc.vector.tensor_scalar(out=h16[:, j, :], in0=iota_b,
                                                    scalar1=mtag_sb[:, i:i + 1],
                                                    scalar2=None, op0=Alu.is_equal)
                        dst = h8[:, g * G:g * G + G, :]
                        if mode == 'AC':
                            nc.scalar.copy(dst, h16)
                        else:
                            nc.gpsimd.tensor_copy(dst, h16)
                    if g == 4:
                        nc.vector.tensor_scalar(out=msk_scr,
                                                in0=mtag_sb[:, 1:KT + 1],
                                                scalar1=128.0, scalar2=0.0,
                                                op0=Alu.is_lt, op1=Alu.add,
                                                accum_out=red[:, 1:2])
                # matmuls trail the one-hot stream by LAG groups
                gm = g - LAG
                if gm < 0:
                    continue
                ntm = G if gm < NEG else 1
                for j in range(ntm):
                    k = gm * G + j - 1
                    if k < 0:
                        continue
                    eg = k // G
                    if eg == 0:
                        e_blkm, e_off = e_tiles[0], 0
                    elif e_tiles[eg] is None:
                        e_blkm, e_off = e_tiles[eg - 1], G
                    else:
                        e_blkm, e_off = e_tiles[eg], 0
                    nc.tensor.matmul(ps_e, lhsT=h8[:, k + 1, :],
                                     rhs=e_blkm[:, e_off + k % G, :],
                                     start=(k == 0), stop=(k == KT - 1),
                                     skip_group_check=True)
                hmax = gm * G + ntm - 1
                while next_ck + 2 <= hmax:
                    kk = next_ck
                    nc.tensor.matmul(ps_c, lhsT=h8[:, kk + 1:kk + 3, :],
                                     rhs=h8[:, kk:kk + 2, :],
                                     start=(kk == 0), stop=(kk == KT - 2),
                                     perf_mode=DR, skip_group_check=True)
                    next_ck += 2

            # ---- final reductions: fused multiply+row-reduce per half ----
            nc.vector.scalar_tensor_tensor(out=scr[:, 128:256], in0=ps_c,
                                           scalar=1.0, in1=idt[:, 128:256],
                                           op0=Alu.mult, op1=Alu.mult,
                                           accum_out=red[:, 2:3])
            nc.vector.scalar_tensor_tensor(out=scr[:, 0:128], in0=ps_e,
                                           scalar=1.0, in1=idt[:, 0:128],
                                           op0=Alu.mult, op1=Alu.mult,
                                           accum_out=red[:, 0:1])
            nc.sync.dma_start(out=out[:, :], in_=red)

    return nc


_nc_cache = None
last_results = None


def _prep_inputs(emissions, tags, mask, transitions):
    import ml_dtypes
    bf16 = ml_dtypes.bfloat16
    em_all = np.ascontiguousarray(emissions.reshape(B * S, T)).astype(bf16)
    tg = tags.reshape(B * S).astype(np.int32)
    mkb = mask.reshape(B * S).astype(np.int32)
    ft = (tg + 128 * (1 - mkb)).astype(bf16)
    trT = np.ascontiguousarray(transitions.astype(np.float32).T)

    p = np.arange(128)
    prevpos = 256 * p - 1
    in_maps = []
    for c in range(NCORES):
        lo = c * NPOS
        t_loc = tg[lo:lo + NPOS]
        m_loc = mkb[lo:lo + NPOS]
        valid = (p % 2 == 1) & (m_loc[prevpos] == 1)
        mt = np.empty((128, KT + 1), dtype=bf16)
        mt[:, 0] = np.where(valid, t_loc[prevpos], 128).astype(bf16)
        mt[:, 1:] = ft[lo:lo + NPOS].reshape(128, KT)
        in_maps.append({
            'em': np.ascontiguousarray(em_all[lo:lo + NPOS]),
            'mtag': mt,
            'tr': trT,
        })
    return in_maps


def kernel(emissions, tags, mask, transitions, _trace=False):
    global _nc_cache, last_results
    from concourse.bass_utils import run_bass_kernel_spmd
    if _nc_cache is None:
        _nc_cache = _build()
    nc = _nc_cache

    in_maps = _prep_inputs(emissions, tags, mask, transitions)
    res = run_bass_kernel_spmd(nc, in_maps, core_ids=list(range(NCORES)),
                               trace=_trace)
    last_results = res
    score = cnt = 0.0
    for r in res.results:
        v = np.asarray(r['out'], dtype=np.float64)
        score += v[:, 0].sum() + v[:, 2].sum()
        cnt += v[:, 1].sum()
    return np.float32(score / cnt)



# revision 11
# speedup vs baseline: 1.2674x; 1.2674x over previous
"""CRF loss kernel v5 for Trainium2: chunked per-element indirect-DMA gather.

The loss touches ~16K emission elements (masked positions) and ~8K
transition elements (valid pairs) per core.  Host computes flat int32
offsets into a concatenated [emissions | transitions | 0.0] DRAM buffer
(index arithmetic on tags/mask only); the device gathers the values with
per-element SWDGE descriptors.

HW semantics (probed): an indirect DMACopy with dest AP [1, J, 1]
(stride-1 middle dim) emits J one-element descriptors, consuming J
offsets from the offsets tile in partition-fastest order, landing
contiguously in partition 0.  Since we only need the SUM of the gathered
values, the placement bijection is irrelevant.  The gather is chunked so
SWDGE descriptor generation (Pool) of chunk i+1 overlaps the DMA
transfer of chunk i; each chunk is then redistributed [1,n] -> [128,
n/128] by a cheap HWDGE copy and row-reduced on DVE while later chunks
are still in flight.  Mask count rides on a parallel path.  Host sums
the [128, R] per-core partials and divides.
"""
import sys
import json

for p in ('/opt/trn_rl_repo', '/opt/trn_rl_repo/concourse'):
    if p not in sys.path:
        sys.path.insert(0, p)

import numpy as np

B, S, T = 512, 512, 128
NCORES = 8
BSH = B // NCORES              # 64 batch rows per core
NPOS = BSH * S                 # 32768 positions per core
M_EM = NPOS * T                # emission elements per core
M_TR = T * T                   # transition table elements
M = M_EM + M_TR + 1            # + trailing 0.0 pad element
MSKC = NPOS // 128             # mask tile free dim (256)

# chunk column-splits (in units of 128 offsets); tuned via TimelineSim.
CHUNK_COLS = [64, 60, 48, 16, 4, 1]    # for C=193; _chunk_plan used otherwise


def _split_waits_json(bir_bytes: bytes, max_waits: int = 1) -> bytes:
    d = json.loads(bir_bytes)
    ctr = 0
    for f in d['functions']:
        for blk in f['blocks']:
            insts = blk.get('instructions')
            if not insts:
                continue
            out = []
            changed = False
            for ins in insts:
                si = ins.get('sync_info')
                if si and len(si.get('on_wait') or []) > max_waits:
                    waits = si['on_wait']
                    for w in waits[:-max_waits]:
                        ctr += 1
                        nop = {'engine': ins['engine'], 'ins': [], 'outs': [],
                               'name': f'wsplit-{ctr}', 'opcode': 'NoOp',
                               'sync_info': {'on_wait': [w], 'on_update': []}}
                        if 'debug' in ins:
                            nop['debug'] = ins['debug']
                        out.append(nop)
                    si['on_wait'] = waits[-max_waits:]
                    changed = True
                out.append(ins)
            if changed:
                blk['instructions'] = out
    return json.dumps(d).encode()


_patched = False


def _install_patch(bass_module):
    global _patched
    if _patched:
        return
    _patched = True
    orig = bass_module.Bass.to_json_bytes

    def patched(self):
        return _split_waits_json(orig(self))

    bass_module.Bass.to_json_bytes = patched


def _chunk_plan(C):
    """Split C columns (128 offsets each) into chunks: small first chunk to
    prime the DMA pipe, large middle chunks (HW cap 8192 descs = 64 cols),
    small last chunk to shorten the tail."""
    CAP = 64
    plan = []
    first = min(8, C)
    plan.append(first)
    rest = C - first
    while rest > 0:
        take = min(CAP, rest)
        # keep a small tail chunk
        if rest - take == 0 and take > 8 and len(plan) >= 1:
            take -= 4
        plan.append(take)
        rest -= take
    return plan


def _build(C, chunk_cols=None):
    import concourse.bass as bass
    import concourse.mybir as mybir
    import concourse.tile as tile
    _install_patch(bass)
    f32 = mybir.dt.float32
    bf16 = mybir.dt.bfloat16
    i32 = mybir.dt.int32

    chunks = chunk_cols if chunk_cols is not None else _chunk_plan(C)
    assert sum(chunks) == C
    K = len(chunks)

    nc = bass.Bass()
    src = nc.dram_tensor('src', [M, 1], f32, kind='ExternalInput')
    off = nc.dram_tensor('off', [128, C], i32, kind='ExternalInput')
    msk = nc.dram_tensor('msk', [128, MSKC], bf16, kind='ExternalInput')
    out = nc.dram_tensor('out', [128, K + 1], f32, kind='ExternalOutput')

    with tile.TileContext(nc) as tc:
        with tc.tile_pool(name='p', bufs=1) as p:
            off_sb = p.tile([128, C], i32)
            msk_sb = p.tile([128, MSKC], bf16)
            red = p.tile([128, K + 1], f32)
            nc.vector.memset(red[:, :], 0.0)

            # per-chunk offset loads (first gates the first gather)
            col = 0
            spans = []
            for n in chunks:
                nc.sync.dma_start(out=off_sb[:, col:col + n],
                                  in_=off[:, col:col + n])
                spans.append((col, n))
                col += n
            nc.sync.dma_start(out=msk_sb, in_=msk[:, :])

            dests = []
            for i, (col, n) in enumerate(spans):
                J = n * 128
                d = p.tile([1, J, 1], f32, tag=f'dest{i}')
                nc.gpsimd.indirect_dma_start(
                    out=d[:, :, :], out_offset=None,
                    in_=src[:, :],
                    in_offset=bass.IndirectOffsetOnAxis(
                        ap=off_sb[:, col:col + n], axis=0))
                dests.append(d)

            nc.vector.tensor_reduce(out=red[:, K:K + 1], in_=msk_sb[:, :],
                                    axis=mybir.AxisListType.X,
                                    op=mybir.AluOpType.add)

            for i, (col, n) in enumerate(spans):
                if n <= 16 and i >= K - 2:
                    # tiny tail chunk: reduce the partition-0 row directly,
                    # skipping the redistribute hop
                    nc.vector.tensor_reduce(out=red[0:1, i:i + 1],
                                            in_=dests[i][:, :, 0],
                                            axis=mybir.AxisListType.X,
                                            op=mybir.AluOpType.add)
                    continue
                d2 = p.tile([128, n], f32, tag=f'd2_{i}')
                eng = nc.scalar if i % 2 else nc.sync
                eng.dma_start(out=d2[:, :], in_=dests[i][:, :, 0])
                nc.vector.tensor_reduce(out=red[:, i:i + 1], in_=d2[:, :],
                                        axis=mybir.AxisListType.X,
                                        op=mybir.AluOpType.add)

            nc.sync.dma_start(out=out[:, :], in_=red)

    return nc


_nc_cache = None
_nc_key = None
last_results = None


def _prep_inputs(emissions, tags, mask, transitions):
    import ml_dtypes
    bf16 = ml_dtypes.bfloat16
    em = np.ascontiguousarray(emissions, dtype=np.float32).reshape(B, S * T)
    tg = np.asarray(tags).astype(np.int32).reshape(B, S)
    mk = np.asarray(mask).astype(bool).reshape(B, S)
    tr = np.ascontiguousarray(transitions, dtype=np.float32).reshape(-1)

    per_core = []
    for c in range(NCORES):
        rows = slice(c * BSH, (c + 1) * BSH)
        tgc = tg[rows]                       # [BSH, S]
        mkc = mk[rows]
        tflat = tgc.ravel()
        mflat = mkc.ravel()
        # emission offsets: pos*T + tag for masked positions
        vpos = np.nonzero(mflat)[0]
        off_e = (vpos * T + tflat[vpos]).astype(np.int32)
        # transition offsets: M_EM + prev*T + cur for valid (in-row) pairs
        pv = mkc[:, 1:] & mkc[:, :-1]
        bb, ss = np.nonzero(pv)
        off_t = (M_EM + tgc[bb, ss] * T + tgc[bb, ss + 1]).astype(np.int32)
        per_core.append((np.concatenate([off_e, off_t]), mflat))

    C = max(1, -(-max(len(o) for o, _ in per_core) // 128))

    in_maps = []
    for c in range(NCORES):
        offs, mflat = per_core[c]
        off_all = np.full(128 * C, M - 1, dtype=np.int32)
        off_all[:len(offs)] = offs
        src = np.empty((M, 1), dtype=np.float32)
        src[:M_EM, 0] = em[c * BSH:(c + 1) * BSH].ravel()
        src[M_EM:M_EM + M_TR, 0] = tr
        src[M - 1, 0] = 0.0
        in_maps.append({
            'src': src,
            'off': off_all.reshape(128, C),
            'msk': mflat.astype(bf16).reshape(128, MSKC),
        })
    return C, in_maps


def kernel(emissions, tags, mask, transitions, _trace=False):
    global _nc_cache, _nc_key, last_results
    from concourse.bass_utils import run_bass_kernel_spmd

    C, in_maps = _prep_inputs(emissions, tags, mask, transitions)
    if _nc_cache is None or _nc_key != C:
        plan = CHUNK_COLS if (CHUNK_COLS and sum(CHUNK_COLS) == C) else None
        _nc_cache = _build(C, plan)
        _nc_key = C
    nc = _nc_cache

    res = run_bass_kernel_spmd(nc, in_maps, core_ids=list(range(NCORES)),
                               trace=_trace)
    last_results = res
    score = cnt = 0.0
    for r in res.results:
        v = np.asarray(r['out'], dtype=np.float64)
        score += v[:, :-1].sum()
        cnt += v[:, -1].sum()
    return np.float32(score / cnt)


# revision 15
# speedup vs baseline: 1.2747x; 1.0058x over previous
"""CRF loss kernel v5 for Trainium2: chunked per-element indirect-DMA gather.

The loss touches ~16K emission elements (masked positions) and ~8K
transition elements (valid pairs) per core.  Host computes flat int32
offsets into a concatenated [emissions | transitions | 0.0] DRAM buffer
(index arithmetic on tags/mask only); the device gathers the values with
per-element SWDGE descriptors.

HW semantics (probed): an indirect DMACopy with dest AP [1, J, 1]
(stride-1 middle dim) emits J one-element descriptors, consuming J
offsets from the offsets tile in partition-fastest order, landing
contiguously in partition 0.  Since we only need the SUM of the gathered
values, the placement bijection is irrelevant.  The gather is chunked so
SWDGE descriptor generation (Pool) of chunk i+1 overlaps the DMA
transfer of chunk i; each chunk is then redistributed [1,n] -> [128,
n/128] by a cheap HWDGE copy and row-reduced on DVE while later chunks
are still in flight.  Mask count rides on a parallel path.  Host sums
the [128, R] per-core partials and divides.
"""
import sys
import json

for p in ('/opt/trn_rl_repo', '/opt/trn_rl_repo/concourse'):
    if p not in sys.path:
        sys.path.insert(0, p)

import numpy as np

B, S, T = 512, 512, 128
NCORES = 8
BSH = B // NCORES              # 64 batch rows per core
NPOS = BSH * S                 # 32768 positions per core
M_EM = NPOS * T                # emission elements per core
M_TR = T * T                   # transition table elements
M = M_EM + M_TR + 1            # + trailing 0.0 pad element
MSKC = NPOS // 128             # mask tile free dim (256)

# chunk column-splits (in units of 128 offsets); tuned via TimelineSim.
CHUNK_COLS = [64, 64, 49, 14, 2]    # for C=193; _chunk_plan used otherwise


def _split_waits_json(bir_bytes: bytes, max_waits: int = 1) -> bytes:
    d = json.loads(bir_bytes)
    ctr = 0
    for f in d['functions']:
        for blk in f['blocks']:
            insts = blk.get('instructions')
            if not insts:
                continue
            out = []
            changed = False
            for ins in insts:
                si = ins.get('sync_info')
                if si and len(si.get('on_wait') or []) > max_waits:
                    waits = si['on_wait']
                    for w in waits[:-max_waits]:
                        ctr += 1
                        nop = {'engine': ins['engine'], 'ins': [], 'outs': [],
                               'name': f'wsplit-{ctr}', 'opcode': 'NoOp',
                               'sync_info': {'on_wait': [w], 'on_update': []}}
                        if 'debug' in ins:
                            nop['debug'] = ins['debug']
                        out.append(nop)
                    si['on_wait'] = waits[-max_waits:]
                    changed = True
                out.append(ins)
            if changed:
                blk['instructions'] = out
    return json.dumps(d).encode()


_patched = False


def _install_patch(bass_module):
    global _patched
    if _patched:
        return
    _patched = True
    orig = bass_module.Bass.to_json_bytes

    def patched(self):
        return _split_waits_json(orig(self))

    bass_module.Bass.to_json_bytes = patched


def _chunk_plan(C):
    """Split C columns (128 offsets each) into chunks: small first chunk to
    prime the DMA pipe, large middle chunks (HW cap 8192 descs = 64 cols),
    small last chunk to shorten the tail."""
    CAP = 64
    plan = []
    first = min(8, C)
    plan.append(first)
    rest = C - first
    while rest > 0:
        take = min(CAP, rest)
        # keep a small tail chunk
        if rest - take == 0 and take > 8 and len(plan) >= 1:
            take -= 4
        plan.append(take)
        rest -= take
    return plan


def _build(C, chunk_cols=None):
    import concourse.bass as bass
    import concourse.mybir as mybir
    import concourse.tile as tile
    _install_patch(bass)
    f32 = mybir.dt.float32
    bf16 = mybir.dt.bfloat16
    i32 = mybir.dt.int32

    chunks = chunk_cols if chunk_cols is not None else _chunk_plan(C)
    assert sum(chunks) == C
    K = len(chunks)

    nc = bass.Bass()
    src = nc.dram_tensor('src', [M, 1], f32, kind='ExternalInput')
    off = nc.dram_tensor('off', [128, C], i32, kind='ExternalInput')
    msk = nc.dram_tensor('msk', [128, MSKC], bf16, kind='ExternalInput')
    out = nc.dram_tensor('out', [128, K + 1], f32, kind='ExternalOutput')

    with tile.TileContext(nc) as tc:
        with tc.tile_pool(name='p', bufs=1) as p:
            off_sb = p.tile([128, C], i32)
            msk_sb = p.tile([128, MSKC], bf16)
            red = p.tile([128, K + 1], f32)
            nc.vector.memset(red[:, :], 0.0)

            # per-chunk offset loads (first gates the first gather)
            col = 0
            spans = []
            for n in chunks:
                nc.sync.dma_start(out=off_sb[:, col:col + n],
                                  in_=off[:, col:col + n])
                spans.append((col, n))
                col += n
            nc.sync.dma_start(out=msk_sb, in_=msk[:, :])

            dests = []
            for i, (col, n) in enumerate(spans):
                J = n * 128
                d = p.tile([1, J, 1], f32, tag=f'dest{i}')
                nc.gpsimd.indirect_dma_start(
                    out=d[:, :, :], out_offset=None,
                    in_=src[:, :],
                    in_offset=bass.IndirectOffsetOnAxis(
                        ap=off_sb[:, col:col + n], axis=0))
                dests.append(d)

            nc.vector.tensor_reduce(out=red[:, K:K + 1], in_=msk_sb[:, :],
                                    axis=mybir.AxisListType.X,
                                    op=mybir.AluOpType.add)

            # tail chunks skip the redistribute hop: partition-0 rows are
            # reduced directly -- the very small ones on DVE, the larger
            # second-to-last one on the otherwise-idle Act engine
            p0_dve = [i for i, (col, n) in enumerate(spans)
                      if n <= 4 and i >= K - 2]
            p0_act = [i for i, (col, n) in enumerate(spans)
                      if 4 < n <= 16 and i >= K - 3 and i not in p0_dve]
            p0 = set(p0_dve) | set(p0_act)
            # redistribute [1,n*128] partition-0 rows into [128,n] tiles
            d2s = {}
            for i, (col, n) in enumerate(spans):
                if i in p0:
                    continue
                d2 = p.tile([128, n], f32, tag=f'd2_{i}')
                eng = nc.scalar if i % 2 else nc.sync
                eng.dma_start(out=d2[:, :], in_=dests[i][:, :, 0])
                d2s[i] = d2
            with tc.high_priority():
                for i in p0_act:
                    n = spans[i][1]
                    scr = p.tile([1, n * 128], f32, tag=f'scr{i}')
                    nc.scalar.activation(out=scr[:, :], in_=dests[i][:, :, 0],
                                         func=mybir.ActivationFunctionType.Copy,
                                         accum_out=red[0:1, i:i + 1])
                for i in p0_dve:
                    nc.vector.tensor_reduce(out=red[0:1, i:i + 1],
                                            in_=dests[i][:, :, 0],
                                            axis=mybir.AxisListType.X,
                                            op=mybir.AluOpType.add)
            for i, (col, n) in enumerate(spans):
                if i in p0:
                    continue
                nc.vector.tensor_reduce(out=red[:, i:i + 1], in_=d2s[i][:, :],
                                        axis=mybir.AxisListType.X,
                                        op=mybir.AluOpType.add)

            nc.sync.dma_start(out=out[:, :], in_=red)

    return nc


_nc_cache = None
_nc_key = None
last_results = None


def _prep_inputs(emissions, tags, mask, transitions):
    import ml_dtypes
    bf16 = ml_dtypes.bfloat16
    em = np.ascontiguousarray(emissions, dtype=np.float32).reshape(B, S * T)
    tg = np.asarray(tags).astype(np.int32).reshape(B, S)
    mk = np.asarray(mask).astype(bool).reshape(B, S)
    tr = np.ascontiguousarray(transitions, dtype=np.float32).reshape(-1)

    per_core = []
    for c in range(NCORES):
        rows = slice(c * BSH, (c + 1) * BSH)
        tgc = tg[rows]                       # [BSH, S]
        mkc = mk[rows]
        tflat = tgc.ravel()
        mflat = mkc.ravel()
        # emission offsets: pos*T + tag for masked positions
        vpos = np.nonzero(mflat)[0]
        off_e = (vpos * T + tflat[vpos]).astype(np.int32)
        # transition offsets: M_EM + prev*T + cur for valid (in-row) pairs
        pv = mkc[:, 1:] & mkc[:, :-1]
        bb, ss = np.nonzero(pv)
        off_t = (M_EM + tgc[bb, ss] * T + tgc[bb, ss + 1]).astype(np.int32)
        per_core.append((np.concatenate([off_e, off_t]), mflat))

    C = max(1, -(-max(len(o) for o, _ in per_core) // 128))

    in_maps = []
    for c in range(NCORES):
        offs, mflat = per_core[c]
        off_all = np.full(128 * C, M - 1, dtype=np.int32)
        off_all[:len(offs)] = offs
        src = np.empty((M, 1), dtype=np.float32)
        src[:M_EM, 0] = em[c * BSH:(c + 1) * BSH].ravel()
        src[M_EM:M_EM + M_TR, 0] = tr
        src[M - 1, 0] = 0.0
        in_maps.append({
            'src': src,
            'off': off_all.reshape(128, C),
            'msk': mflat.astype(bf16).reshape(128, MSKC),
        })
    return C, in_maps


def kernel(emissions, tags, mask, transitions, _trace=False):
    global _nc_cache, _nc_key, last_results
    from concourse.bass_utils import run_bass_kernel_spmd

    C, in_maps = _prep_inputs(emissions, tags, mask, transitions)
    if _nc_cache is None or _nc_key != C:
        plan = CHUNK_COLS if (CHUNK_COLS and sum(CHUNK_COLS) == C) else None
        _nc_cache = _build(C, plan)
        _nc_key = C
    nc = _nc_cache

    res = run_bass_kernel_spmd(nc, in_maps, core_ids=list(range(NCORES)),
                               trace=_trace)
    last_results = res
    score = cnt = 0.0
    for r in res.results:
        v = np.asarray(r['out'], dtype=np.float64)
        score += v[:, :-1].sum()
        cnt += v[:, -1].sum()
    return np.float32(score / cnt)


# revision 17
# speedup vs baseline: 1.2792x; 1.0035x over previous
"""CRF loss kernel v5 for Trainium2: chunked per-element indirect-DMA gather.

The loss touches ~16K emission elements (masked positions) and ~8K
transition elements (valid pairs) per core.  Host computes flat int32
offsets into a concatenated [emissions | transitions | 0.0] DRAM buffer
(index arithmetic on tags/mask only); the device gathers the values with
per-element SWDGE descriptors.

HW semantics (probed): an indirect DMACopy with dest AP [1, J, 1]
(stride-1 middle dim) emits J one-element descriptors, consuming J
offsets from the offsets tile in partition-fastest order, landing
contiguously in partition 0.  Since we only need the SUM of the gathered
values, the placement bijection is irrelevant.  The gather is chunked so
SWDGE descriptor generation (Pool) of chunk i+1 overlaps the DMA
transfer of chunk i; each chunk is then redistributed [1,n] -> [128,
n/128] by a cheap HWDGE copy and row-reduced on DVE while later chunks
are still in flight.  Mask count rides on a parallel path.  Host sums
the [128, R] per-core partials and divides.
"""
import sys
import json

for p in ('/opt/trn_rl_repo', '/opt/trn_rl_repo/concourse'):
    if p not in sys.path:
        sys.path.insert(0, p)

import numpy as np

B, S, T = 512, 512, 128
NCORES = 8
BSH = B // NCORES              # 64 batch rows per core
NPOS = BSH * S                 # 32768 positions per core
M_EM = NPOS * T                # emission elements per core
M_TR = T * T                   # transition table elements
M = M_EM + M_TR + 1            # + trailing 0.0 pad element
MSKC = NPOS // 128             # mask tile free dim (256)

# chunk column-splits (in units of 128 offsets); tuned via TimelineSim.
CHUNK_COLS = [64, 64, 49, 8, 4, 4]    # for C=193; _chunk_plan used otherwise


def _split_waits_json(bir_bytes: bytes, max_waits: int = 1) -> bytes:
    d = json.loads(bir_bytes)
    ctr = 0
    for f in d['functions']:
        for blk in f['blocks']:
            insts = blk.get('instructions')
            if not insts:
                continue
            out = []
            changed = False
            for ins in insts:
                si = ins.get('sync_info')
                if si and len(si.get('on_wait') or []) > max_waits:
                    waits = si['on_wait']
                    for w in waits[:-max_waits]:
                        ctr += 1
                        nop = {'engine': ins['engine'], 'ins': [], 'outs': [],
                               'name': f'wsplit-{ctr}', 'opcode': 'NoOp',
                               'sync_info': {'on_wait': [w], 'on_update': []}}
                        if 'debug' in ins:
                            nop['debug'] = ins['debug']
                        out.append(nop)
                    si['on_wait'] = waits[-max_waits:]
                    changed = True
                out.append(ins)
            if changed:
                blk['instructions'] = out
    return json.dumps(d).encode()


_patched = False


def _install_patch(bass_module):
    global _patched
    if _patched:
        return
    _patched = True
    orig = bass_module.Bass.to_json_bytes

    def patched(self):
        return _split_waits_json(orig(self))

    bass_module.Bass.to_json_bytes = patched


def _chunk_plan(C):
    """Split C columns (128 offsets each) into chunks: small first chunk to
    prime the DMA pipe, large middle chunks (HW cap 8192 descs = 64 cols),
    small last chunk to shorten the tail."""
    CAP = 64
    plan = []
    first = min(8, C)
    plan.append(first)
    rest = C - first
    while rest > 0:
        take = min(CAP, rest)
        # keep a small tail chunk
        if rest - take == 0 and take > 8 and len(plan) >= 1:
            take -= 4
        plan.append(take)
        rest -= take
    return plan


def _build(C, chunk_cols=None):
    import concourse.bass as bass
    import concourse.mybir as mybir
    import concourse.tile as tile
    _install_patch(bass)
    f32 = mybir.dt.float32
    bf16 = mybir.dt.bfloat16
    i32 = mybir.dt.int32

    chunks = chunk_cols if chunk_cols is not None else _chunk_plan(C)
    assert sum(chunks) == C
    K = len(chunks)

    nc = bass.Bass()
    src = nc.dram_tensor('src', [M, 1], f32, kind='ExternalInput')
    off = nc.dram_tensor('off', [128, C], i32, kind='ExternalInput')
    msk = nc.dram_tensor('msk', [128, MSKC], bf16, kind='ExternalInput')
    out = nc.dram_tensor('out', [128, K + 1], f32, kind='ExternalOutput')

    with tile.TileContext(nc) as tc:
        with tc.tile_pool(name='p', bufs=1) as p:
            off_sb = p.tile([128, C], i32)
            msk_sb = p.tile([128, MSKC], bf16)
            red = p.tile([128, K + 1], f32)
            nc.vector.memset(red[:, :], 0.0)

            # per-chunk offset loads (first gates the first gather)
            col = 0
            spans = []
            for n in chunks:
                nc.sync.dma_start(out=off_sb[:, col:col + n],
                                  in_=off[:, col:col + n])
                spans.append((col, n))
                col += n
            nc.sync.dma_start(out=msk_sb, in_=msk[:, :])

            dests = []
            for i, (col, n) in enumerate(spans):
                J = n * 128
                d = p.tile([1, J, 1], f32, tag=f'dest{i}')
                nc.gpsimd.indirect_dma_start(
                    out=d[:, :, :], out_offset=None,
                    in_=src[:, :],
                    in_offset=bass.IndirectOffsetOnAxis(
                        ap=off_sb[:, col:col + n], axis=0))
                dests.append(d)

            nc.vector.tensor_reduce(out=red[:, K:K + 1], in_=msk_sb[:, :],
                                    axis=mybir.AxisListType.X,
                                    op=mybir.AluOpType.add)

            # tail chunks skip the redistribute hop: partition-0 rows are
            # reduced directly -- the very small ones on DVE, the larger
            # second-to-last one on the otherwise-idle Act engine
            p0_dve = [i for i, (col, n) in enumerate(spans)
                      if n <= 4 and i >= K - 2]
            p0_act = [i for i, (col, n) in enumerate(spans)
                      if 4 < n <= 16 and i >= K - 3 and i not in p0_dve]
            p0 = set(p0_dve) | set(p0_act)
            redist = [i for i in range(K) if i not in p0]
            r_last = max(redist) if redist else None
            # redistribute [1,n*128] partition-0 rows into [128,n] tiles
            d2s = {}
            for i in redist:
                n = spans[i][1]
                d2 = p.tile([128, n], f32, tag=f'd2_{i}')
                nc.sync.dma_start(out=d2[:, :], in_=dests[i][:, :, 0])
                d2s[i] = d2
            # early redist reduces on DVE (deps fire before the p0 deps, so
            # the DVE in-order queue never stalls the tail)
            for i in redist:
                if i == r_last:
                    continue
                nc.vector.tensor_reduce(out=red[:, i:i + 1], in_=d2s[i][:, :],
                                        axis=mybir.AxisListType.X,
                                        op=mybir.AluOpType.add)
            for i in p0_dve:
                nc.vector.tensor_reduce(out=red[0:1, i:i + 1],
                                        in_=dests[i][:, :, 0],
                                        axis=mybir.AxisListType.X,
                                        op=mybir.AluOpType.add)
            # Act chain: p0_act first (data ready earlier), then the last
            # redistributed chunk's reduce -- keeps the late dependency off
            # the DVE queue entirely
            for i in p0_act:
                n = spans[i][1]
                scr = p.tile([1, n * 128], f32, tag=f'scr{i}')
                nc.scalar.activation(out=scr[:, :], in_=dests[i][:, :, 0],
                                     func=mybir.ActivationFunctionType.Copy,
                                     accum_out=red[0:1, i:i + 1])
            if r_last is not None:
                n = spans[r_last][1]
                scr2 = p.tile([128, n], f32, tag='scr_rl')
                nc.scalar.activation(out=scr2[:, :], in_=d2s[r_last][:, :],
                                     func=mybir.ActivationFunctionType.Copy,
                                     accum_out=red[:, r_last:r_last + 1])

            nc.sync.dma_start(out=out[:, :], in_=red)

    return nc


_nc_cache = None
_nc_key = None
last_results = None


def _prep_inputs(emissions, tags, mask, transitions):
    import ml_dtypes
    bf16 = ml_dtypes.bfloat16
    em = np.ascontiguousarray(emissions, dtype=np.float32).reshape(B, S * T)
    tg = np.asarray(tags).astype(np.int32).reshape(B, S)
    mk = np.asarray(mask).astype(bool).reshape(B, S)
    tr = np.ascontiguousarray(transitions, dtype=np.float32).reshape(-1)

    per_core = []
    for c in range(NCORES):
        rows = slice(c * BSH, (c + 1) * BSH)
        tgc = tg[rows]                       # [BSH, S]
        mkc = mk[rows]
        tflat = tgc.ravel()
        mflat = mkc.ravel()
        # emission offsets: pos*T + tag for masked positions
        vpos = np.nonzero(mflat)[0]
        off_e = (vpos * T + tflat[vpos]).astype(np.int32)
        # transition offsets: M_EM + prev*T + cur for valid (in-row) pairs
        pv = mkc[:, 1:] & mkc[:, :-1]
        bb, ss = np.nonzero(pv)
        off_t = (M_EM + tgc[bb, ss] * T + tgc[bb, ss + 1]).astype(np.int32)
        per_core.append((np.concatenate([off_e, off_t]), mflat))

    C = max(1, -(-max(len(o) for o, _ in per_core) // 128))

    in_maps = []
    for c in range(NCORES):
        offs, mflat = per_core[c]
        off_all = np.full(128 * C, M - 1, dtype=np.int32)
        off_all[:len(offs)] = offs
        src = np.empty((M, 1), dtype=np.float32)
        src[:M_EM, 0] = em[c * BSH:(c + 1) * BSH].ravel()
        src[M_EM:M_EM + M_TR, 0] = tr
        src[M - 1, 0] = 0.0
        in_maps.append({
            'src': src,
            'off': off_all.reshape(128, C),
            'msk': mflat.astype(bf16).reshape(128, MSKC),
        })
    return C, in_maps


def kernel(emissions, tags, mask, transitions, _trace=False):
    global _nc_cache, _nc_key, last_results
    from concourse.bass_utils import run_bass_kernel_spmd

    C, in_maps = _prep_inputs(emissions, tags, mask, transitions)
    if _nc_cache is None or _nc_key != C:
        plan = CHUNK_COLS if (CHUNK_COLS and sum(CHUNK_COLS) == C) else None
        _nc_cache = _build(C, plan)
        _nc_key = C
    nc = _nc_cache

    res = run_bass_kernel_spmd(nc, in_maps, core_ids=list(range(NCORES)),
                               trace=_trace)
    last_results = res
    score = cnt = 0.0
    for r in res.results:
        v = np.asarray(r['out'], dtype=np.float64)
        score += v[:, :-1].sum()
        cnt += v[:, -1].sum()
    return np.float32(score / cnt)


# revision 18
# speedup vs baseline: 1.2946x; 1.0120x over previous
"""CRF loss kernel v5 for Trainium2: chunked per-element indirect-DMA gather.

The loss touches ~16K emission elements (masked positions) and ~8K
transition elements (valid pairs) per core.  Host computes flat int32
offsets into a concatenated [emissions | transitions | 0.0] DRAM buffer
(index arithmetic on tags/mask only); the device gathers the values with
per-element SWDGE descriptors.

HW semantics (probed): an indirect DMACopy with dest AP [1, J, 1]
(stride-1 middle dim) emits J one-element descriptors, consuming J
offsets from the offsets tile in partition-fastest order, landing
contiguously in partition 0.  Since we only need the SUM of the gathered
values, the placement bijection is irrelevant.  The gather is chunked so
SWDGE descriptor generation (Pool) of chunk i+1 overlaps the DMA
transfer of chunk i; each chunk is then redistributed [1,n] -> [128,
n/128] by a cheap HWDGE copy and row-reduced on DVE while later chunks
are still in flight.  Mask count rides on a parallel path.  Host sums
the [128, R] per-core partials and divides.
"""
import sys
import json

for p in ('/opt/trn_rl_repo', '/opt/trn_rl_repo/concourse'):
    if p not in sys.path:
        sys.path.insert(0, p)

import numpy as np

B, S, T = 512, 512, 128
NCORES = 8
BSH = B // NCORES              # 64 batch rows per core
NPOS = BSH * S                 # 32768 positions per core
M_EM = NPOS * T                # emission elements per core
M_TR = T * T                   # transition table elements
M = M_EM + M_TR + 1            # + trailing 0.0 pad element
MSKC = NPOS // 128             # mask tile free dim (256)

# chunk column-splits (in units of 128 offsets); tuned via TimelineSim.
CHUNK_COLS = [64, 58, 53, 10, 4, 4]    # for C=193; _chunk_plan used otherwise


def _split_waits_json(bir_bytes: bytes, max_waits: int = 1) -> bytes:
    d = json.loads(bir_bytes)
    ctr = 0
    for f in d['functions']:
        for blk in f['blocks']:
            insts = blk.get('instructions')
            if not insts:
                continue
            out = []
            changed = False
            for ins in insts:
                si = ins.get('sync_info')
                if si and len(si.get('on_wait') or []) > max_waits:
                    waits = si['on_wait']
                    for w in waits[:-max_waits]:
                        ctr += 1
                        nop = {'engine': ins['engine'], 'ins': [], 'outs': [],
                               'name': f'wsplit-{ctr}', 'opcode': 'NoOp',
                               'sync_info': {'on_wait': [w], 'on_update': []}}
                        if 'debug' in ins:
                            nop['debug'] = ins['debug']
                        out.append(nop)
                    si['on_wait'] = waits[-max_waits:]
                    changed = True
                out.append(ins)
            if changed:
                blk['instructions'] = out
    return json.dumps(d).encode()


_patched = False


def _install_patch(bass_module):
    global _patched
    if _patched:
        return
    _patched = True
    orig = bass_module.Bass.to_json_bytes

    def patched(self):
        return _split_waits_json(orig(self))

    bass_module.Bass.to_json_bytes = patched


def _chunk_plan(C):
    """Split C columns (128 offsets each) into chunks: small first chunk to
    prime the DMA pipe, large middle chunks (HW cap 8192 descs = 64 cols),
    small last chunk to shorten the tail."""
    CAP = 64
    plan = []
    first = min(8, C)
    plan.append(first)
    rest = C - first
    while rest > 0:
        take = min(CAP, rest)
        # keep a small tail chunk
        if rest - take == 0 and take > 8 and len(plan) >= 1:
            take -= 4
        plan.append(take)
        rest -= take
    return plan


def _build(C, chunk_cols=None):
    import concourse.bass as bass
    import concourse.mybir as mybir
    import concourse.tile as tile
    _install_patch(bass)
    f32 = mybir.dt.float32
    bf16 = mybir.dt.bfloat16
    i32 = mybir.dt.int32

    chunks = chunk_cols if chunk_cols is not None else _chunk_plan(C)
    assert sum(chunks) == C
    K = len(chunks)

    nc = bass.Bass()
    src = nc.dram_tensor('src', [M, 1], f32, kind='ExternalInput')
    off = nc.dram_tensor('off', [128, C], i32, kind='ExternalInput')
    msk = nc.dram_tensor('msk', [128, MSKC], bf16, kind='ExternalInput')
    out = nc.dram_tensor('out', [128, K + 1], f32, kind='ExternalOutput')

    with tile.TileContext(nc) as tc:
        with tc.tile_pool(name='p', bufs=1) as p:
            off_sb = p.tile([128, C], i32)
            msk_sb = p.tile([128, MSKC], bf16)
            red = p.tile([128, K + 1], f32)
            nc.vector.memset(red[:, :], 0.0)

            # per-chunk offset loads (first gates the first gather)
            col = 0
            spans = []
            for n in chunks:
                nc.sync.dma_start(out=off_sb[:, col:col + n],
                                  in_=off[:, col:col + n])
                spans.append((col, n))
                col += n
            nc.sync.dma_start(out=msk_sb, in_=msk[:, :])

            dests = []
            for i, (col, n) in enumerate(spans):
                J = n * 128
                d = p.tile([1, J, 1], f32, tag=f'dest{i}')
                nc.gpsimd.indirect_dma_start(
                    out=d[:, :, :], out_offset=None,
                    in_=src[:, :],
                    in_offset=bass.IndirectOffsetOnAxis(
                        ap=off_sb[:, col:col + n], axis=0))
                dests.append(d)

            nc.vector.tensor_reduce(out=red[:, K:K + 1], in_=msk_sb[:, :],
                                    axis=mybir.AxisListType.X,
                                    op=mybir.AluOpType.add)

            # tail chunks skip the redistribute hop: partition-0 rows are
            # reduced directly -- the very small ones on DVE, the larger
            # second-to-last one on the otherwise-idle Act engine
            p0_dve = [i for i, (col, n) in enumerate(spans)
                      if n <= 4 and i >= K - 2]
            p0_act = [i for i, (col, n) in enumerate(spans)
                      if 4 < n <= 16 and i >= K - 3 and i not in p0_dve]
            p0 = set(p0_dve) | set(p0_act)
            redist = [i for i in range(K) if i not in p0]
            r_last = max(redist) if redist else None
            # redistribute [1,n*128] partition-0 rows into [128,n] tiles
            d2s = {}
            for i in redist:
                n = spans[i][1]
                d2 = p.tile([128, n], f32, tag=f'd2_{i}')
                nc.sync.dma_start(out=d2[:, :], in_=dests[i][:, :, 0])
                d2s[i] = d2
            # early redist reduces on DVE (deps fire before the p0 deps, so
            # the DVE in-order queue never stalls the tail)
            for i in redist:
                if i == r_last:
                    continue
                nc.vector.tensor_reduce(out=red[:, i:i + 1], in_=d2s[i][:, :],
                                        axis=mybir.AxisListType.X,
                                        op=mybir.AluOpType.add)
            for i in p0_dve:
                nc.vector.tensor_reduce(out=red[0:1, i:i + 1],
                                        in_=dests[i][:, :, 0],
                                        axis=mybir.AxisListType.X,
                                        op=mybir.AluOpType.add)
            # Act chain: p0_act first (data ready earlier), then the last
            # redistributed chunk's reduce -- keeps the late dependency off
            # the DVE queue entirely
            for i in p0_act:
                n = spans[i][1]
                scr = p.tile([1, n * 128], f32, tag=f'scr{i}')
                nc.scalar.activation(out=scr[:, :], in_=dests[i][:, :, 0],
                                     func=mybir.ActivationFunctionType.Copy,
                                     accum_out=red[0:1, i:i + 1])
            if r_last is not None:
                n = spans[r_last][1]
                scr2 = p.tile([128, n], f32, tag='scr_rl')
                nc.scalar.activation(out=scr2[:, :], in_=d2s[r_last][:, :],
                                     func=mybir.ActivationFunctionType.Copy,
                                     accum_out=red[:, r_last:r_last + 1])

            nc.sync.dma_start(out=out[:, :], in_=red)

    return nc


_nc_cache = None
_nc_key = None
last_results = None


def _prep_inputs(emissions, tags, mask, transitions):
    import ml_dtypes
    bf16 = ml_dtypes.bfloat16
    em = np.ascontiguousarray(emissions, dtype=np.float32).reshape(B, S * T)
    tg = np.asarray(tags).astype(np.int32).reshape(B, S)
    mk = np.asarray(mask).astype(bool).reshape(B, S)
    tr = np.ascontiguousarray(transitions, dtype=np.float32).reshape(-1)

    per_core = []
    for c in range(NCORES):
        rows = slice(c * BSH, (c + 1) * BSH)
        tgc = tg[rows]                       # [BSH, S]
        mkc = mk[rows]
        tflat = tgc.ravel()
        mflat = mkc.ravel()
        # emission offsets: pos*T + tag for masked positions
        vpos = np.nonzero(mflat)[0]
        off_e = (vpos * T + tflat[vpos]).astype(np.int32)
        # transition offsets: M_EM + prev*T + cur for valid (in-row) pairs
        pv = mkc[:, 1:] & mkc[:, :-1]
        bb, ss = np.nonzero(pv)
        off_t = (M_EM + tgc[bb, ss] * T + tgc[bb, ss + 1]).astype(np.int32)
        per_core.append((np.concatenate([off_e, off_t]), mflat))

    C = max(1, -(-max(len(o) for o, _ in per_core) // 128))

    in_maps = []
    for c in range(NCORES):
        offs, mflat = per_core[c]
        off_all = np.full(128 * C, M - 1, dtype=np.int32)
        off_all[:len(offs)] = offs
        src = np.empty((M, 1), dtype=np.float32)
        src[:M_EM, 0] = em[c * BSH:(c + 1) * BSH].ravel()
        src[M_EM:M_EM + M_TR, 0] = tr
        src[M - 1, 0] = 0.0
        in_maps.append({
            'src': src,
            'off': off_all.reshape(128, C),
            'msk': mflat.astype(bf16).reshape(128, MSKC),
        })
    return C, in_maps


def kernel(emissions, tags, mask, transitions, _trace=False):
    global _nc_cache, _nc_key, last_results
    from concourse.bass_utils import run_bass_kernel_spmd

    C, in_maps = _prep_inputs(emissions, tags, mask, transitions)
    if _nc_cache is None or _nc_key != C:
        plan = CHUNK_COLS if (CHUNK_COLS and sum(CHUNK_COLS) == C) else None
        _nc_cache = _build(C, plan)
        _nc_key = C
    nc = _nc_cache

    res = run_bass_kernel_spmd(nc, in_maps, core_ids=list(range(NCORES)),
                               trace=_trace)
    last_results = res
    score = cnt = 0.0
    for r in res.results:
        v = np.asarray(r['out'], dtype=np.float64)
        score += v[:, :-1].sum()
        cnt += v[:, -1].sum()
    return np.float32(score / cnt)


# revision 20
# speedup vs baseline: 1.3046x; 1.0077x over previous
"""CRF loss kernel v5 for Trainium2: chunked per-element indirect-DMA gather.

The loss touches ~16K emission elements (masked positions) and ~8K
transition elements (valid pairs) per core.  Host computes flat int32
offsets into a concatenated [emissions | transitions | 0.0] DRAM buffer
(index arithmetic on tags/mask only); the device gathers the values with
per-element SWDGE descriptors.

HW semantics (probed): an indirect DMACopy with dest AP [1, J, 1]
(stride-1 middle dim) emits J one-element descriptors, consuming J
offsets from the offsets tile in partition-fastest order, landing
contiguously in partition 0.  Since we only need the SUM of the gathered
values, the placement bijection is irrelevant.  The gather is chunked so
SWDGE descriptor generation (Pool) of chunk i+1 overlaps the DMA
transfer of chunk i; each chunk is then redistributed [1,n] -> [128,
n/128] by a cheap HWDGE copy and row-reduced on DVE while later chunks
are still in flight.  Mask count rides on a parallel path.  Host sums
the [128, R] per-core partials and divides.
"""
import sys
import json

for p in ('/opt/trn_rl_repo', '/opt/trn_rl_repo/concourse'):
    if p not in sys.path:
        sys.path.insert(0, p)

import numpy as np

B, S, T = 512, 512, 128
NCORES = 8
BSH = B // NCORES              # 64 batch rows per core
NPOS = BSH * S                 # 32768 positions per core
M_EM = NPOS * T                # emission elements per core
M_TR = T * T                   # transition table elements
M = M_EM + M_TR + 1            # + trailing 0.0 pad element
MSKC = NPOS // 128             # mask tile free dim (256)

# chunk column-splits (in units of 128 offsets); tuned via TimelineSim.
CHUNK_COLS = [64, 59, 52, 10, 4, 4]    # for C=193; _chunk_plan used otherwise


def _split_waits_json(bir_bytes: bytes, max_waits: int = 1) -> bytes:
    d = json.loads(bir_bytes)
    ctr = 0
    for f in d['functions']:
        for blk in f['blocks']:
            insts = blk.get('instructions')
            if not insts:
                continue
            out = []
            changed = False
            for ins in insts:
                si = ins.get('sync_info')
                if si and len(si.get('on_wait') or []) > max_waits:
                    waits = si['on_wait']
                    for w in waits[:-max_waits]:
                        ctr += 1
                        nop = {'engine': ins['engine'], 'ins': [], 'outs': [],
                               'name': f'wsplit-{ctr}', 'opcode': 'NoOp',
                               'sync_info': {'on_wait': [w], 'on_update': []}}
                        if 'debug' in ins:
                            nop['debug'] = ins['debug']
                        out.append(nop)
                    si['on_wait'] = waits[-max_waits:]
                    changed = True
                out.append(ins)
            if changed:
                blk['instructions'] = out
    return json.dumps(d).encode()


_patched = False


def _install_patch(bass_module):
    global _patched
    if _patched:
        return
    _patched = True
    orig = bass_module.Bass.to_json_bytes

    def patched(self):
        return _split_waits_json(orig(self))

    bass_module.Bass.to_json_bytes = patched


def _chunk_plan(C):
    """Split C columns (128 offsets each) into chunks: small first chunk to
    prime the DMA pipe, large middle chunks (HW cap 8192 descs = 64 cols),
    small last chunk to shorten the tail."""
    CAP = 64
    plan = []
    first = min(8, C)
    plan.append(first)
    rest = C - first
    while rest > 0:
        take = min(CAP, rest)
        # keep a small tail chunk
        if rest - take == 0 and take > 8 and len(plan) >= 1:
            take -= 4
        plan.append(take)
        rest -= take
    return plan


def _build(C, chunk_cols=None):
    import concourse.bass as bass
    import concourse.mybir as mybir
    import concourse.tile as tile
    _install_patch(bass)
    f32 = mybir.dt.float32
    bf16 = mybir.dt.bfloat16
    i32 = mybir.dt.int32

    chunks = chunk_cols if chunk_cols is not None else _chunk_plan(C)
    assert sum(chunks) == C
    K = len(chunks)

    nc = bass.Bass()
    src = nc.dram_tensor('src', [M, 1], f32, kind='ExternalInput')
    off = nc.dram_tensor('off', [128, C], i32, kind='ExternalInput')
    msk = nc.dram_tensor('msk', [128, MSKC], bf16, kind='ExternalInput')
    out = nc.dram_tensor('out', [128, K + 1], f32, kind='ExternalOutput')

    with tile.TileContext(nc) as tc:
        with tc.tile_pool(name='p', bufs=1) as p:
            off_sb = p.tile([128, C], i32)
            msk_sb = p.tile([128, MSKC], bf16)
            red = p.tile([128, K + 1], f32)
            nc.vector.memset(red[:, :], 0.0)

            # per-chunk offset loads (first gates the first gather)
            col = 0
            spans = []
            for n in chunks:
                nc.sync.dma_start(out=off_sb[:, col:col + n],
                                  in_=off[:, col:col + n])
                spans.append((col, n))
                col += n
            nc.sync.dma_start(out=msk_sb, in_=msk[:, :])

            dests = []
            for i, (col, n) in enumerate(spans):
                J = n * 128
                d = p.tile([1, J, 1], f32, tag=f'dest{i}')
                nc.gpsimd.indirect_dma_start(
                    out=d[:, :, :], out_offset=None,
                    in_=src[:, :],
                    in_offset=bass.IndirectOffsetOnAxis(
                        ap=off_sb[:, col:col + n], axis=0))
                dests.append(d)

            nc.vector.tensor_reduce(out=red[:, K:K + 1], in_=msk_sb[:, :],
                                    axis=mybir.AxisListType.X,
                                    op=mybir.AluOpType.add)

            # tail chunks skip the redistribute hop: partition-0 rows are
            # reduced directly -- the very small ones on DVE, the larger
            # second-to-last one on the otherwise-idle Act engine
            p0_dve = [i for i, (col, n) in enumerate(spans)
                      if n <= 4 and i >= K - 2]
            p0_act = [i for i, (col, n) in enumerate(spans)
                      if 4 < n <= 16 and i >= K - 3 and i not in p0_dve]
            p0 = set(p0_dve) | set(p0_act)
            redist = [i for i in range(K) if i not in p0]
            r_last = max(redist) if redist else None
            # redistribute [1,n*128] partition-0 rows into [128,n] tiles.
            # The first one goes through the Pool SWDGE queue: it then runs
            # after the gather gens instead of stealing a DMA-engine slot in
            # the middle of the gather transfer train.
            d2s = {}
            for i in redist:
                n = spans[i][1]
                d2 = p.tile([128, n], f32, tag=f'd2_{i}')
                if i == redist[0]:
                    nc.gpsimd.dma_start(out=d2[:, :], in_=dests[i][:, :, 0])
                else:
                    nc.sync.dma_start(out=d2[:, :], in_=dests[i][:, :, 0])
                d2s[i] = d2
            # early redist reduces on DVE (deps fire before the p0 deps, so
            # the DVE in-order queue never stalls the tail)
            for i in redist:
                if i == r_last:
                    continue
                nc.vector.tensor_reduce(out=red[:, i:i + 1], in_=d2s[i][:, :],
                                        axis=mybir.AxisListType.X,
                                        op=mybir.AluOpType.add)
            for i in p0_dve:
                nc.vector.tensor_reduce(out=red[0:1, i:i + 1],
                                        in_=dests[i][:, :, 0],
                                        axis=mybir.AxisListType.X,
                                        op=mybir.AluOpType.add)
            # Act chain: p0_act first (data ready earlier), then the last
            # redistributed chunk's reduce -- keeps the late dependency off
            # the DVE queue entirely
            for i in p0_act:
                n = spans[i][1]
                scr = p.tile([1, n * 128], f32, tag=f'scr{i}')
                nc.scalar.activation(out=scr[:, :], in_=dests[i][:, :, 0],
                                     func=mybir.ActivationFunctionType.Copy,
                                     accum_out=red[0:1, i:i + 1])
            if r_last is not None:
                n = spans[r_last][1]
                scr2 = p.tile([128, n], f32, tag='scr_rl')
                nc.scalar.activation(out=scr2[:, :], in_=d2s[r_last][:, :],
                                     func=mybir.ActivationFunctionType.Copy,
                                     accum_out=red[:, r_last:r_last + 1])

            nc.sync.dma_start(out=out[:, :], in_=red)

    return nc


_nc_cache = None
_nc_key = None
last_results = None


def _prep_inputs(emissions, tags, mask, transitions):
    import ml_dtypes
    bf16 = ml_dtypes.bfloat16
    em = np.ascontiguousarray(emissions, dtype=np.float32).reshape(B, S * T)
    tg = np.asarray(tags).astype(np.int32).reshape(B, S)
    mk = np.asarray(mask).astype(bool).reshape(B, S)
    tr = np.ascontiguousarray(transitions, dtype=np.float32).reshape(-1)

    per_core = []
    for c in range(NCORES):
        rows = slice(c * BSH, (c + 1) * BSH)
        tgc = tg[rows]                       # [BSH, S]
        mkc = mk[rows]
        tflat = tgc.ravel()
        mflat = mkc.ravel()
        # emission offsets: pos*T + tag for masked positions
        vpos = np.nonzero(mflat)[0]
        off_e = (vpos * T + tflat[vpos]).astype(np.int32)
        # transition offsets: M_EM + prev*T + cur for valid (in-row) pairs
        pv = mkc[:, 1:] & mkc[:, :-1]
        bb, ss = np.nonzero(pv)
        off_t = (M_EM + tgc[bb, ss] * T + tgc[bb, ss + 1]).astype(np.int32)
        per_core.append((np.concatenate([off_e, off_t]), mflat))

    C = max(1, -(-max(len(o) for o, _ in per_core) // 128))

    in_maps = []
    for c in range(NCORES):
        offs, mflat = per_core[c]
        off_all = np.full(128 * C, M - 1, dtype=np.int32)
        off_all[:len(offs)] = offs
        src = np.empty((M, 1), dtype=np.float32)
        src[:M_EM, 0] = em[c * BSH:(c + 1) * BSH].ravel()
        src[M_EM:M_EM + M_TR, 0] = tr
        src[M - 1, 0] = 0.0
        in_maps.append({
            'src': src,
            'off': off_all.reshape(128, C),
            'msk': mflat.astype(bf16).reshape(128, MSKC),
        })
    return C, in_maps


def kernel(emissions, tags, mask, transitions, _trace=False):
    global _nc_cache, _nc_key, last_results
    from concourse.bass_utils import run_bass_kernel_spmd

    C, in_maps = _prep_inputs(emissions, tags, mask, transitions)
    if _nc_cache is None or _nc_key != C:
        plan = CHUNK_COLS if (CHUNK_COLS and sum(CHUNK_COLS) == C) else None
        _nc_cache = _build(C, plan)
        _nc_key = C
    nc = _nc_cache

    res = run_bass_kernel_spmd(nc, in_maps, core_ids=list(range(NCORES)),
                               trace=_trace)
    last_results = res
    score = cnt = 0.0
    for r in res.results:
        v = np.asarray(r['out'], dtype=np.float64)
        score += v[:, :-1].sum()
        cnt += v[:, -1].sum()
    return np.float32(score / cnt)
